# revision 26
# baseline (speedup 1.0000x reference)
"""Trainium2 Bass kernel: single-head causal attention (v2.4).

Problem: x[4,2048,1024] f32; q/k/v = x@W* + b* (head dim 128);
out = softmax(causal(q k^T / sqrt(128))) @ v.

Sharding: 8 cores = 4 batches x 2 causal "wedges". Within a batch, the 16
query blocks (128 rows each) are interleaved between the two cores
(h=0 takes odd global blocks, h=1 even) so both cores carry an identical
static schedule: slot p attends exactly L_p = 2p+2 local key blocks.
Per-core key order is a host-side permutation (h=0 identity, h=1
adjacent-pair swap) that puts slot p's own (diagonal) block at local
position 2p+1; the wedge difference is carried by a mask input, so a
single NEFF serves all 8 cores (SPMD).

Pipeline (per core):
  - q/k projections in fp8 e4m3 with DoubleRow (contraction 256/matmul);
    weights prescaled x8 (all values normal), 1/8 folded into the
    psum->sbuf copy scale. The fp8 x stream is DMA'd first so the
    ACT-bound S^T/exp pipeline starts well before the bf16 stream ends.
    fp8 x columns are host-reordered: own-query blocks first, so the q
    projection reads a contiguous slice (4D DR APs lower wrong) and kT
    gets a position remap (kpos) for S^T.
  - v projection in bf16 (fp8 v breaks the 2e-2 budget), shipped in 4
    column-groups of 512 keys so v^T groups complete progressively;
    early groups are transposed off-PE via the XBAR DMA transpose
    (latency hidden behind S^T/exp), late groups on the PE (shorter
    latency chain at the tail).
  - PV bursts (slot p over its 2p+2 key blocks, denominator via the v
    ones-column) interleave with S^T as groups complete.
  - k/q psum->sbuf affine copies split across ACT and DVE to halve the
    S^T unblock latency.
  - All bulk tensors partition-major (4KB contiguous runs per partition;
    DMA is element-rate limited ~210 Gelem/s and small runs halve it).
"""

import numpy as np

B, T, D, DK = 4, 2048, 1024, 128
NBLK = T // 128      # 16 key blocks per core
NSLOT = 8            # q slots per core (NSLOT*128 = 1024 q rows)
NCHUNK = D // 128    # bf16 m-chunks (v projection)
NDC = D // 256       # fp8 double-chunks (q/k projections)
NG = 4               # v column groups (512 keys each)
GW = T // NG         # group width (512)
SCALE = 1.0 / np.sqrt(np.float32(DK))
WS = 8.0             # fp8 weight prescale (power of 2; undone in psum copy)
WARMUP_MMS = 10
N_XBAR_G = 2         # groups 0..N_XBAR_G-1 via XBAR DMA transpose, rest PE

_built = None


def _build():
    from contextlib import ExitStack

    import concourse.bass as bass
    import concourse.mybir as mybir
    import concourse.tile as tile
    from concourse import bacc
    from concourse.masks import make_identity

    f32 = mybir.dt.float32
    bf16 = mybir.dt.bfloat16
    fp8 = mybir.dt.float8e4
    Act = mybir.ActivationFunctionType
    Alu = mybir.AluOpType
    DR = mybir.MatmulPerfMode.DoubleRow

    nc = bacc.Bacc("TRN2", target_bir_lowering=False, debug=False, num_devices=8)

    # all bulk inputs partition-major: [128, ...] with >=4KB contiguous runs
    x8p = nc.dram_tensor("x8p", [128, NDC * 2 * T], fp8, kind="ExternalInput").ap()
    xgp = nc.dram_tensor("xgp", [128, NG * NCHUNK * GW], bf16,
                         kind="ExternalInput").ap()
    wk8 = nc.dram_tensor("wk8", [128, NDC * 2 * DK], fp8, kind="ExternalInput").ap()
    wq8 = nc.dram_tensor("wq8", [128, NDC * 2 * DK], fp8, kind="ExternalInput").ap()
    wv = nc.dram_tensor("wv", [128, NCHUNK * DK], bf16, kind="ExternalInput").ap()
    # biases packed [128, 3]: col 0 = bq, col 1 = bk*SCALE, col 2 = bv
    bias = nc.dram_tensor("bias", [DK, 3], f32, kind="ExternalInput").ap()
    masks = nc.dram_tensor("masks", [128, 256], bf16, kind="ExternalInput").ap()
    o = nc.dram_tensor("o", [NSLOT * 128, DK], f32, kind="ExternalOutput").ap()

    with tile.TileContext(nc) as tc, ExitStack() as ctx:
        const = ctx.enter_context(tc.tile_pool(name="const", bufs=1))
        sbufs = ctx.enter_context(tc.tile_pool(name="sbufs", bufs=1))
        x8_pool = ctx.enter_context(tc.tile_pool(name="x8_pool", bufs=NDC))
        xg_pool = ctx.enter_context(tc.tile_pool(name="xg_pool", bufs=NG))
        vs_pool = ctx.enter_context(tc.tile_pool(name="vs_pool", bufs=NG))
        stg_pool = ctx.enter_context(tc.tile_pool(name="stg_pool", bufs=2))
        out_pool = ctx.enter_context(tc.tile_pool(name="out_pool", bufs=3))

        # ---- DMAs in priority order: fp8 q/k stream first, then bf16 v groups
        wk8_sb = const.tile([128, NDC, 2, DK], fp8, tag="wk8")
        nc.sync.dma_start(out=wk8_sb, in_=wk8)

        x8s = []
        for dc in range(NDC):
            x8t = x8_pool.tile([128, 2, T], fp8, tag="x8", name=f"x8_{dc}")
            x8s.append(x8t)

        def load_x8(dc):
            nc.sync.dma_start(
                out=x8s[dc], in_=x8p[:, 2 * T * dc : 2 * T * (dc + 1)]
            )

        load_x8(0)
        wq8_sb = const.tile([128, NDC, 2, DK], fp8, tag="wq8")
        nc.sync.dma_start(out=wq8_sb, in_=wq8)
        load_x8(1)
        xgs = [None] * NG

        def load_xg(g):
            xg = xg_pool.tile([128, NCHUNK, GW], bf16, tag="xg", name=f"xg{g}")
            xgs[g] = xg
            nc.sync.dma_start(
                out=xg, in_=xgp[:, NCHUNK * GW * g : NCHUNK * GW * (g + 1)]
            )

        load_xg(0)
        bias_sb = const.tile([128, 3], f32, tag="bias")
        nc.sync.dma_start(out=bias_sb, in_=bias)
        bq_sb = bias_sb[:, 0:1]
        bks_sb = bias_sb[:, 1:2]
        bv_sb = bias_sb[:, 2:3]
        load_x8(2)
        load_x8(3)
        mask_sb = const.tile([128, 256], bf16, tag="mask")
        nc.sync.dma_start(out=mask_sb, in_=masks)
        wv_sb = const.tile([128, NCHUNK * DK], bf16, tag="wv")
        nc.sync.dma_start(out=wv_sb, in_=wv)

        load_xg(1)
        load_xg(2)
        load_xg(3)

        ident = const.tile([128, 128], bf16, tag="ident")
        make_identity(nc, ident)
        # v in natural [k, v] layout, bf16, with a ones column appended
        v_aug = const.tile([128, NBLK, DK + 1], bf16, tag="vaug")
        nc.vector.memset(v_aug[:, :, DK : DK + 1], 1.0)

        # ---- PE warmup: bridge the DMA-wait window at kernel start; the exp
        # ACT_TABLE_LOAD (~1.3us) is pulled out of the attention phase.
        with tc.tile_pool(name="warmps", bufs=1, space="PSUM") as warmps:
            wsrc = sbufs.tile([128, 512], bf16, tag="wsrc")
            nc.vector.memset(wsrc, 0.0)
            wdst = warmps.tile([128, 512], f32, tag="warm")
            for _ in range(WARMUP_MMS):
                nc.tensor.matmul(
                    wdst, lhsT=wsrc[:, 0:128], rhs=wsrc, start=True, stop=True
                )
            wexp = sbufs.tile([128, 1], f32, tag="wexp")
            nc.scalar.activation(out=wexp, in_=wsrc[:, 0:1], func=Act.Exp, scale=1.0)

        # ---- q/k projections (fp8 DoubleRow, contraction 256 per matmul).
        # x8 columns host-reordered: own-query blocks (odd locals, slot order)
        # in cols 0:1024, even locals in cols 1024:2048.
        # halves as separate tiles: tile-granular dep tracking lets the first
        # S^T matmuls start after 2 copies instead of all 6
        kT_lo = sbufs.tile([128, T // 2], bf16, tag="kTl")  # positions 0..7
        kT_hi = sbufs.tile([128, T // 2], bf16, tag="kTh")  # positions 8..15
        qT_lo = sbufs.tile([128, 512], bf16, tag="qTl")     # slots 0..3
        qT_hi = sbufs.tile([128, 512], bf16, tag="qTh")     # slots 4..7

        vpool = tc.alloc_tile_pool(name="vpool", bufs=2, space="PSUM")
        kpool = tc.alloc_tile_pool(name="kpool", bufs=1, space="PSUM")
        qpool = tc.alloc_tile_pool(name="qpool", bufs=1, space="PSUM")

        def emit_v_group(g):
            """8 bf16 matmuls -> vg psum [128,512] -> +bv -> v^T sbuf slice."""
            vg_ps = vpool.tile([128, GW], f32, tag="vps", name=f"vg{g}")
            for c in range(NCHUNK):
                nc.tensor.matmul(
                    vg_ps,
                    lhsT=wv_sb[:, 128 * c : 128 * (c + 1)],
                    rhs=xgs[g][:, c, :],
                    start=(c == 0),
                    stop=(c == NCHUNK - 1),
                )
            vg_sb = vs_pool.tile([128, GW], bf16, tag="vg", name=f"vgs{g}")
            nc.vector.tensor_scalar_add(vg_sb, vg_ps, bv_sb)
            return vg_sb

        def emit_transpose_xbar(g, vg_sb):
            """whole group via XBAR DMA: [dk,512] -> [key,4,dk] staging -> v_aug."""
            stg = stg_pool.tile([128, 4, DK], bf16, tag="stg", name=f"stg{g}")
            nc.sync.dma_start_transpose(stg, vg_sb)
            nc.vector.tensor_copy(v_aug[:, 4 * g : 4 * g + 4, 0:DK], stg)

        def emit_transpose_pe(g, vg_sb):
            for b in range(4):
                j = 4 * g + b
                vt_ps = opool.tile([128, DK + 1], bf16, tag="o", name=f"vt{j}")
                nc.tensor.transpose(
                    vt_ps[:, 0:128], vg_sb[:, 128 * b : 128 * (b + 1)], ident
                )
                nc.vector.tensor_copy(v_aug[:, j, 0:DK], vt_ps[:, 0:128])

        def emit_group(g):
            vg_sb = emit_v_group(g)
            if g < N_XBAR_G:
                emit_transpose_xbar(g, vg_sb)
            else:
                emit_transpose_pe(g, vg_sb)

        kT_ps = kpool.tile([128, T], f32, tag="kps")
        qT_ps = qpool.tile([128, NSLOT * 128], f32, tag="qps")
        kq_dcs = [0, 1, None, 2, 3]   # None = slot where v group 0 is emitted
        for dc in kq_dcs:
            if dc is None:
                emit_group(0)
                continue
            for t in range(4):
                nc.tensor.matmul(
                    kT_ps[:, 512 * t : 512 * (t + 1)],
                    lhsT=wk8_sb[:, dc, :, :],
                    rhs=x8s[dc][:, :, 512 * t : 512 * (t + 1)],
                    start=(dc == 0),
                    stop=(dc == NDC - 1),
                    perf_mode=DR,
                )
            for t in range(2):
                nc.tensor.matmul(
                    qT_ps[:, 512 * t : 512 * (t + 1)],
                    lhsT=wq8_sb[:, dc, :, :],
                    rhs=x8s[dc][:, :, 512 * t : 512 * (t + 1)],
                    start=(dc == 0),
                    stop=(dc == NDC - 1),
                    perf_mode=DR,
                )
        # psum->sbuf affine copies, split ACT (kT_lo + qT) / DVE (kT_hi)
        nc.scalar.activation(
            out=kT_lo[:, 0:512], in_=kT_ps[:, 0:512], func=Act.Identity,
            bias=bks_sb, scale=SCALE / WS,
        )
        nc.scalar.activation(
            out=qT_lo, in_=qT_ps[:, 0:512], func=Act.Identity,
            bias=bq_sb, scale=1.0 / WS,
        )
        nc.scalar.activation(
            out=qT_hi, in_=qT_ps[:, 512:1024], func=Act.Identity,
            bias=bq_sb, scale=1.0 / WS,
        )
        nc.scalar.activation(
            out=kT_lo[:, 512:1024], in_=kT_ps[:, 512:1024], func=Act.Identity,
            bias=bks_sb, scale=SCALE / WS,
        )
        for t in range(2):
            sl = slice(512 * t, 512 * (t + 1))
            nc.vector.tensor_scalar(
                out=kT_hi[:, sl], in0=kT_ps[:, 1024 + 512 * t : 1024 + 512 * (t + 1)],
                scalar1=float(SCALE / WS), scalar2=bks_sb,
                op0=Alu.mult, op1=Alu.add,
            )
        qpool.release()
        kpool.release()

        # ---- attention: S^T/exp, v groups, transposes, PV bursts ----
        pt_pool = ctx.enter_context(tc.tile_pool(name="pt_pool", bufs=NBLK))
        spool = tc.alloc_tile_pool(name="spool", bufs=3, space="PSUM")
        opool = tc.alloc_tile_pool(name="opool", bufs=3, space="PSUM")

        def chunk_sizes(n):
            out = []
            while n > 512:
                out.append(512)
                n -= 512
            out.append(n)
            return out

        pts = [None] * NBLK

        def kpos(j):
            # column position of local key block j in the reordered x8/kT
            return (j - 1) // 2 if j % 2 == 1 else NSLOT + j // 2

        def emit_st(j):
            sj = j // 2           # first active slot for this key position
            q0 = 128 * sj
            qn = NSLOT * 128 - q0
            pt = pt_pool.tile([128, qn], bf16, tag="pt", name=f"pt{j}")
            pts[j] = pt
            kp = kpos(j)
            kt = kT_lo if kp < NSLOT else kT_hi
            kp = kp % NSLOT
            # chunks split at the qT_lo/qT_hi boundary (col 512)
            if q0 < 512:
                pieces = [(qT_lo, q0, 512 - q0), (qT_hi, 0, 512)]
            else:
                pieces = [(qT_hi, q0 - 512, 1024 - q0)]
            off = 0
            for qtile, qoff, sz in pieces:
                s_ps = spool.tile([128, 512], f32, tag="st")
                nc.tensor.matmul(
                    s_ps[:, :sz],
                    lhsT=kt[:, 128 * kp : 128 * kp + 128],
                    rhs=qtile[:, qoff : qoff + sz],
                    start=True,
                    stop=True,
                )
                nc.scalar.activation(
                    out=pt[:, off : off + sz], in_=s_ps[:, :sz], func=Act.Exp,
                    scale=1.0,
                )
                if off == 0:
                    # mask the frontier slot multiplicatively (exp(s+m) =
                    # exp(s)*m01): even j -> maskA (wedge-dependent), odd j ->
                    # maskB (causal triangle)
                    sel = j % 2
                    nc.vector.tensor_mul(
                        pt[:, 0:128],
                        pt[:, 0:128],
                        mask_sb[:, 128 * sel : 128 * (sel + 1)],
                    )
                off += sz

        def emit_burst(p):
            o_ps = opool.tile([128, DK + 1], f32, tag="o", name=f"o_ps{p}")
            for jj in range(2 * p + 2):
                nc.tensor.matmul(
                    o_ps,
                    lhsT=pts[jj][:, 128 * (p - jj // 2) : 128 * (p - jj // 2) + 128],
                    rhs=v_aug[:, jj, :],
                    start=(jj == 0),
                    stop=(jj == 2 * p + 1),
                )
            rcp = out_pool.tile([128, 1], f32, tag="rcp")
            nc.vector.reciprocal(rcp, o_ps[:, DK : DK + 1])
            ob = out_pool.tile([128, DK], f32, tag="ob")
            nc.vector.tensor_scalar_mul(ob, o_ps[:, 0:DK], rcp)
            nc.sync.dma_start(out=o[128 * p : 128 * (p + 1), :], in_=ob)

        # emission = PE execution order; data-arrival pacing:
        #   x8 dc0 ~11us, xg groups progressively until ~29us
        emit_st(1); emit_st(0)
        emit_burst(0)
        emit_st(3); emit_st(2)
        emit_burst(1)
        emit_st(5); emit_st(4)
        emit_group(1)
        emit_burst(2)
        emit_st(7); emit_st(6)
        emit_burst(3)
        emit_st(9); emit_st(8)
        emit_group(2)
        emit_burst(4)
        emit_group(3)
        emit_st(11); emit_st(10)
        emit_burst(5)
        emit_st(13); emit_st(12)
        emit_st(15); emit_st(14)
        emit_burst(6)
        emit_burst(7)

        opool.release()
        spool.release()
        vpool.release()

    nc.compile()
    return nc


def get_built():
    global _built
    if _built is None:
        _built = _build()
    return _built


def _pos2glob(h):
    if h == 0:
        return list(range(NBLK))
    return [j + 1 if j % 2 == 0 else j - 1 for j in range(NBLK)]


def _pack_w_bf16(W):
    """[D, DK] -> [128, NCHUNK*DK] with column block c holding rows 128c..."""
    import ml_dtypes
    return np.ascontiguousarray(
        np.asarray(W, np.float32).reshape(NCHUNK, 128, DK).transpose(1, 0, 2)
        .reshape(128, NCHUNK * DK).astype(ml_dtypes.bfloat16)
    )


def _pack_w_fp8(W):
    """[D, DK] -> [128, NDC*2*DK] e4m3: [p, ((dc*2+i)*DK)+d] = e4m3(WS*W[256dc+128i+p, d])."""
    import ml_dtypes
    Ws = np.asarray(W, np.float32) * WS
    return np.ascontiguousarray(
        Ws.reshape(NDC, 2, 128, DK).transpose(2, 0, 1, 3)
        .reshape(128, NDC * 2 * DK).astype(ml_dtypes.float8_e4m3)
    )


def make_in_map(x_b, Wq, bq, Wk, bk, Wv, bv, h, xT_pre=None, x8T_pre=None):
    """Build one core's input dict. x_b: [T, D] fp32 for this core's batch.
    xT_pre/x8T_pre: optional precomputed transposed/cast copies (shared by
    both wedge cores of a batch; h=0 uses as-is, h=1 column-permutes)."""
    import ml_dtypes
    bf = ml_dtypes.bfloat16
    if xT_pre is None:
        xT_pre = np.ascontiguousarray(x_b.T.astype(bf))
    if x8T_pre is None:
        x8T_pre = np.ascontiguousarray(x_b.T.astype(ml_dtypes.float8_e4m3))
    if h == 0:
        xT_loc, x8T_loc = xT_pre, x8T_pre
    else:
        p2g = _pos2glob(h)
        cols = np.concatenate([np.arange(128 * g, 128 * (g + 1)) for g in p2g])
        xT_loc = np.ascontiguousarray(xT_pre[:, cols])
        x8T_loc = np.ascontiguousarray(x8T_pre[:, cols])
    # x8 column order: own-query blocks (odd locals, slot order) first, then
    # the even locals -- q projection reads cols 0:1024 contiguously
    korder = list(range(1, NBLK, 2)) + list(range(0, NBLK, 2))
    qcols = np.concatenate([np.arange(128 * j, 128 * (j + 1)) for j in korder])
    # x8p[p, (dc*2+i)*T + t] = x8T[256dc+128i+p, perm(t)]  (partition-major)
    x8p = np.ascontiguousarray(
        x8T_loc[:, qcols].reshape(NDC, 2, 128, T).transpose(2, 0, 1, 3)
        .reshape(128, NDC * 2 * T)
    )
    # xgp[p, (g*NCHUNK+c)*GW + t'] = xT[128c+p, GW*g+t']  (partition-major)
    xgp = np.ascontiguousarray(
        xT_loc.reshape(NCHUNK, 128, NG, GW).transpose(1, 2, 0, 3)
        .reshape(128, NG * NCHUNK * GW)
    )
    maskA = (np.ones if h == 0 else np.zeros)((128, 128), bf)
    kk = np.arange(128)
    maskB = np.where(kk[:, None] <= kk[None, :], 1.0, 0.0).astype(bf)
    bias = np.stack([
        np.asarray(bq, np.float32),
        np.asarray(bk, np.float32) * SCALE,
        np.asarray(bv, np.float32),
    ], axis=1)
    return {
        "x8p": x8p,
        "xgp": xgp,
        "wk8": _pack_w_fp8(Wk),
        "wq8": _pack_w_fp8(Wq),
        "wv": _pack_w_bf16(Wv),
        "bias": np.ascontiguousarray(bias),
        "masks": np.ascontiguousarray(np.concatenate([maskA, maskB], axis=1)),
    }


def gather_out(results):
    """results: list of 8 dicts with 'o' [1024, 128] -> full [B, T, DK]."""
    out = np.zeros((B, T, DK), np.float32)
    for core in range(8):
        b, h = core // 2, core % 2
        ob = results[core]["o"]
        for p in range(NSLOT):
            g = 2 * p + 1 - h
            out[b, 128 * g : 128 * (g + 1), :] = ob[128 * p : 128 * (p + 1), :]
    return out


def kernel(x, Wq, bq, Wk, bk, Wv, bv):
    import ml_dtypes
    from concourse.bass_utils import run_bass_kernel_spmd

    x = np.asarray(x, np.float32)
    args = [np.asarray(a, np.float32) for a in (Wq, bq, Wk, bk, Wv, bv)]
    nc = get_built()
    # one transpose+cast per batch, shared by its two wedge cores
    xT_pres = [np.ascontiguousarray(x[b].T.astype(ml_dtypes.bfloat16))
               for b in range(B)]
    x8T_pres = [np.ascontiguousarray(x[b].T.astype(ml_dtypes.float8_e4m3))
                for b in range(B)]
    in_maps = [
        make_in_map(x[core // 2], args[0], args[1], args[2], args[3], args[4],
                    args[5], core % 2, xT_pre=xT_pres[core // 2],
                    x8T_pre=x8T_pres[core // 2])
        for core in range(8)
    ]
    res = run_bass_kernel_spmd(nc, in_maps, core_ids=list(range(8)))
    return gather_out(res.results)


if __name__ == "__main__":
    rng = np.random.default_rng(0)
    x = rng.standard_normal((B, T, D), dtype=np.float32)
    Wq = rng.standard_normal((D, DK), dtype=np.float32) * 0.03
    out = kernel(x, Wq, np.zeros(DK, np.float32), Wq, np.zeros(DK, np.float32),
                 Wq, np.zeros(DK, np.float32))
    print(out.shape)
